# revision 23
# baseline (speedup 1.0000x reference)
"""Trainium2 Bass kernel for nn_BatchedCauchyKernel (v5c).

Computes, for x[N,D], y[M,D], sample_x[N,S], sample_y[M,S], scale[S]:
    d[i,j] = |x_i|^2 + |y_j|^2 - 2 x_i.y_j
    sx_i   = clip(softplus(sample_x_i . scale), 1e-10, 1e4);  u_i = sx_i^-1/2
    sy_j   = clip(softplus(sample_y_j . scale), 1e-10, 1e4);  v_j = sy_j^-1/2
    res    = 1 / (1 + u_i v_j d[i,j])
    out    = res * sigmoid(phi * (res - clip(cutoff, 0, 1000)))

Sharding: 2D grid over 8 cores, 4 x-blocks (NS=2048) x 2 y-blocks (MS=2048).

Device program per core (tile-granular PSUM dep tracking: all writes of a
psum tile are emitted before its single wide reader):
    denom*(1/C) = 1/C + (1/(256 C)) * T_ij
    T = 256*[ u sqx v + u v sqy - 2 u v dot ] accumulated per [128,2048]
    psum tile as 8 fp8-DoubleRow passes (x8 = 16*u_i*x, y8 = -32*v_j*y)
    plus one K=6 bf16 extension pass per 512-quarter carrying the
    u*sqx (x) 256v  and  u (x) 256*v*sqy  rank-1 terms (hi/lo split).
    out = Reciprocal(T/(256 C) + 1/C) on ACT (immediate scale+bias,
    one 2048-wide activation per tile), written bf16.
    Mask sigmoid folded into C (runtime-verified const fit; linear
    fallback multiplies (res + c0)*res on DVE).
Warmup matmuls ramp the PE clock while input DMA is in flight; inputs
are chunked/need-ordered across the two HWDGE queues.
"""

import os
import sys

sys.path.insert(0, "/opt/trn_rl_repo")

import numpy as np

N, M, D, S = 8192, 4096, 512, 16
XB, YB = 4, 2  # core grid
CORES = XB * YB
NS = N // XB  # 2048 rows of x per core
MS = M // YB  # 2048 cols (y rows) per core
PO = NS // 128  # 16 i-tiles
KT = D // 128  # 4 k-tiles
JT = MS // 512  # 4 j-quadrants per psum tile

SOFTPLUS_MIN = 1e-10
SOFTPLUS_MAX = 10000.0

GX = 16.0       # fp8 scale for x rows
GY = -32.0      # fp8 scale for y rows (carries the -2)
PSC = 256.0     # resulting P scale: P = 256 * (-2 u v dot)

_CACHE = {}

N_WARMUP = int(os.environ.get("N_WARMUP", "6"))


def _act_recip(nc, out, in_, scale, bias):
    """out = 1/(scale*in + bias) on the ACT engine (immediate scale/bias)."""
    import concourse.mybir as mybir

    eng = nc.scalar
    inputs = [eng.lower_ap(in_)]
    for val in (bias, scale, 0.0):
        inputs.append(
            mybir.ImmediateValue(dtype=mybir.dt.float32, value=float(val))
        )
    return eng.add_instruction(
        mybir.InstActivation(
            name=nc.get_next_instruction_name(),
            func=mybir.ActivationFunctionType.Reciprocal,
            ins=inputs,
            outs=[eng.lower_ap(out)],
        )
    )


def _fit_mask(phi_val, cutoff_val, R):
    """Linear + constant fits of sigmoid(phi*(t-c)) on [0,R]."""
    t = (np.cos(np.linspace(0, np.pi, 2001)) + 1) * (R / 2)
    g = 1.0 / (1.0 + np.exp(-phi_val * (t - cutoff_val)))
    m1_, m0_ = np.polyfit(t, g, 1)
    gerr = np.abs(np.polyval([m1_, m0_], t) - g) / np.abs(g)
    gmin, gmax = g.min(), g.max()
    c_const = 2.0 * gmin * gmax / (gmin + gmax)
    const_err = (gmax - gmin) / (gmax + gmin)
    return float(m0_), float(m1_), float(gerr.max()), float(c_const), float(const_err)


def _build(const_mask: bool, c0: float, bias: float, scale: float):
    import concourse.mybir as mybir
    import concourse.tile as tile
    from concourse import bacc

    dt = mybir.dt
    OP = mybir.AluOpType
    PM = mybir.MatmulPerfMode

    nc = bacc.Bacc("TRN2", target_bir_lowering=False)

    # chunked input layouts: per partition [kt2][grp][ktpair][512]
    x8_d = nc.dram_tensor("x8T_shard", [128, KT * NS], dt.float8e4,
                          kind="ExternalInput")
    y8_d = nc.dram_tensor("y8T_shard", [128, KT * MS], dt.float8e4,
                          kind="ExternalInput")
    extl_d = nc.dram_tensor("extL_shard", [6, NS], dt.bfloat16,
                            kind="ExternalInput")
    extr_d = nc.dram_tensor("extR_shard", [6, MS], dt.bfloat16,
                            kind="ExternalInput")
    out_d = nc.dram_tensor("out_shard", [NS, MS], dt.bfloat16,
                           kind="ExternalOutput")

    x8_v = x8_d.rearrange("p (kt2 g ktp i) -> p kt2 g ktp i", kt2=2, g=4, ktp=2)
    y8_v = y8_d.rearrange("p (kt2 g ktp j) -> p kt2 g ktp j", kt2=2, g=4, ktp=2)
    out_v = out_d.rearrange("(po pi) j -> pi po j", pi=128)

    with tile.TileContext(nc) as tc:
        with (
            tc.tile_pool(name="persist", bufs=1) as persist,
            tc.tile_pool(name="psum", bufs=2, space="PSUM") as psum_p,
            tc.tile_pool(name="stage", bufs=3) as stage,
        ):
            x8_sb = persist.tile([128, 2, 4, 2, 512], dt.float8e4)
            y8_sb = persist.tile([128, 2, 4, 2, 512], dt.float8e4)
            extl_sb = persist.tile([6, NS], dt.bfloat16)
            extr_sb = persist.tile([6, MS], dt.bfloat16)
            # warmup scratch for p-state ramp (vector is idle early)
            warm_sb = persist.tile([128, 2, 256], dt.float8e4)
            nc.vector.memset(warm_sb[:], 0.0)

            # input DMA, need-ordered on the two HWDGE queues (sync is
            # otherwise idle until the first output at ~12us)
            nc.sync.dma_start(extl_sb[:], extl_d[:, :])
            nc.sync.dma_start(extr_sb[:], extr_d[:, :])
            nc.sync.dma_start(y8_sb[:, 0], y8_v[:, 0])
            nc.sync.dma_start(y8_sb[:, 1], y8_v[:, 1])
            nc.scalar.dma_start(x8_sb[:, :, 0], x8_v[:, :, 0])
            for g in range(1, 4):
                nc.scalar.dma_start(x8_sb[:, :, g], x8_v[:, :, g])

            # warmup matmuls: no data deps; ramp the PE clock during DMA
            for w in range(N_WARMUP):
                wps = psum_p.tile([128, 2048], dt.float32, tag="mm",
                                  name=f"warm{w}")
                nc.tensor.matmul(
                    wps[:, 0:256],
                    lhsT=warm_sb[:, :, 0:128],
                    rhs=warm_sb[:, :, :],
                    start=True, stop=True,
                    perf_mode=PM.DoubleRow,
                )

            for po in range(PO):
                pog, pi = divmod(po, 4)
                lhsT = x8_sb[:, :, pog, :, pi * 128:(pi + 1) * 128]
                pst = psum_p.tile([128, 2048], dt.float32, tag="mm",
                                  name=f"mm{po}")
                ot = stage.tile([128, MS], dt.bfloat16, tag="ot",
                                name=f"ot{po}")
                if not const_mask:
                    rt = stage.tile([128, MS], dt.float32, tag="rt",
                                    name=f"rt{po}")
                last = po == PO - 1

                for jt in range(JT):
                    sl = slice(jt * 512, (jt + 1) * 512)
                    for kt2 in range(2):
                        nc.tensor.matmul(
                            pst[:, sl],
                            lhsT=lhsT[:, kt2],
                            rhs=y8_sb[:, kt2, jt],
                            start=(kt2 == 0), stop=False,
                            perf_mode=PM.DoubleRow,
                        )
                    nc.tensor.matmul(
                        pst[:, sl],
                        lhsT=extl_sb[:, po * 128:(po + 1) * 128],
                        rhs=extr_sb[:, sl],
                        start=False, stop=True,
                    )

                nh = 4 if last else 1
                w = MS // nh
                for h in range(nh):
                    d = slice(h * w, (h + 1) * w)
                    if const_mask:
                        _act_recip(nc, ot[:, d], pst[:, d], scale, bias)
                    else:
                        _act_recip(nc, rt[:, d], pst[:, d], scale, bias)
                        nc.vector.scalar_tensor_tensor(
                            ot[:, d], rt[:, d], c0, rt[:, d],
                            OP.add, OP.mult,
                        )
                    nc.sync.dma_start(out_v[:, po, d], ot[:, d])

    nc.compile()
    return nc


def _hi_lo(vec, bf16):
    hi = vec.astype(bf16)
    lo = (vec - hi.astype(np.float64)).astype(bf16)
    return hi, lo


def kernel(x, y, sample_x, sample_y, scale, cutoff, phi):
    import ml_dtypes
    from concourse.bass_utils import run_bass_kernel_spmd

    bf16 = ml_dtypes.bfloat16
    fp8 = ml_dtypes.float8_e4m3

    phi_val = float(np.asarray(phi).reshape(-1)[0])
    cutoff_val = float(np.clip(np.asarray(cutoff).reshape(-1)[0], 0.0, 1000.0))

    x64 = np.asarray(x, dtype=np.float32).astype(np.float64)
    y64 = np.asarray(y, dtype=np.float32).astype(np.float64)
    sc64 = np.asarray(scale, dtype=np.float32).astype(np.float64).reshape(-1)
    sx64 = np.asarray(sample_x, dtype=np.float32).astype(np.float64)
    sy64 = np.asarray(sample_y, dtype=np.float32).astype(np.float64)

    u = np.clip(np.log1p(np.exp(sx64 @ sc64)), SOFTPLUS_MIN, SOFTPLUS_MAX) ** -0.5
    v = np.clip(np.log1p(np.exp(sy64 @ sc64)), SOFTPLUS_MIN, SOFTPLUS_MAX) ** -0.5
    sqx = (x64 * x64).sum(axis=1)  # [N]
    sqy = (y64 * y64).sum(axis=1)  # [M]

    # res range from a subsample -> mask fit interval
    rng = np.random.default_rng(12345)
    ii = rng.integers(0, N, 4096)
    jj = rng.integers(0, M, 4096)
    dd = sqx[ii] + sqy[jj] - 2.0 * np.einsum("nd,nd->n", x64[ii], y64[jj])
    res_s = 1.0 / (1.0 + dd * (u[ii] * v[jj]))
    R = float(min(1.0, max(2.0 * res_s.max(), 0.01)))

    m0, m1, gerr, c_const, const_err = _fit_mask(phi_val, cutoff_val, R)
    const_mask = const_err < 6e-3
    if const_mask:
        cc = c_const
        c0 = 0.0
    else:
        assert gerr < 2e-3, f"mask linearization too coarse: {gerr}"
        cc = float(np.sqrt(m1))  # ACT produces sqrt(m1)*res; DVE applies mask
        c0 = m0 / cc
    bias = 1.0 / cc
    act_scale = 1.0 / (PSC * cc)

    key = (const_mask, round(c0, 9), round(bias, 9))
    if key not in _CACHE:
        _CACHE[key] = _build(const_mask, c0, bias, act_scale)
    nc = _CACHE[key]

    x8T = (GX * (u[:, None] * x64)).T.astype(fp8)   # [D, N]
    y8T = (GY * (v[:, None] * y64)).T.astype(fp8)   # [D, M]
    wv = PSC * v              # [M]
    brw = PSC * v * sqy       # [M]
    ah, al = _hi_lo(u * sqx, bf16)                  # [N]
    uh, ul = _hi_lo(u, bf16)
    wvh, wvl = _hi_lo(wv, bf16)
    bh, bl = _hi_lo(brw, bf16)
    extL = np.stack([ah, ah, al, uh, uh, ul])
    extR = np.stack([wvh, wvl, wvh, bh, bl, bh])

    in_maps = []
    for c in range(CORES):
        cx, cy = divmod(c, YB)
        si, sj = cx * NS, cy * MS
        xt = x8T[:, si:si + NS].reshape(2, 2, 128, 4, 512)  # [kt2,ktp,p,g,c]
        yt = y8T[:, sj:sj + MS].reshape(2, 2, 128, 4, 512)
        in_maps.append(
            {
                "x8T_shard": np.ascontiguousarray(
                    xt.transpose(2, 0, 3, 1, 4)).reshape(128, KT * NS),
                "y8T_shard": np.ascontiguousarray(
                    yt.transpose(2, 0, 3, 1, 4)).reshape(128, KT * MS),
                "extL_shard": np.ascontiguousarray(extL[:, si:si + NS]),
                "extR_shard": np.ascontiguousarray(extR[:, sj:sj + MS]),
            }
        )

    trace = bool(int(os.environ.get("KERNEL_TRACE", "0")))
    r = run_bass_kernel_spmd(nc, in_maps, core_ids=list(range(CORES)), trace=trace)
    kernel.last_results = r
    out = np.empty((N, M), dtype=np.float32)
    for c in range(CORES):
        cx, cy = divmod(c, YB)
        out[cx * NS:(cx + 1) * NS, cy * MS:(cy + 1) * MS] = np.asarray(
            r.results[c]["out_shard"]
        ).astype(np.float32)
    return out


if __name__ == "__main__":
    rng = np.random.default_rng(0)
    ins = {
        "x": rng.standard_normal((N, D), dtype=np.float32),
        "y": rng.standard_normal((M, D), dtype=np.float32),
        "sample_x": rng.random((N, S), dtype=np.float32),
        "sample_y": rng.random((M, S), dtype=np.float32),
        "scale": rng.random((S,), dtype=np.float32),
        "cutoff": np.full((1,), 0.1, dtype=np.float32),
        "phi": np.ones((1,), dtype=np.float32),
    }
    o = kernel(**ins)
    print(o.shape, o.dtype, o[:2, :4])


# revision 24
# speedup vs baseline: 1.5940x; 1.5940x over previous
"""Trainium2 Bass kernel for nn_BatchedCauchyKernel (v5c).

Computes, for x[N,D], y[M,D], sample_x[N,S], sample_y[M,S], scale[S]:
    d[i,j] = |x_i|^2 + |y_j|^2 - 2 x_i.y_j
    sx_i   = clip(softplus(sample_x_i . scale), 1e-10, 1e4);  u_i = sx_i^-1/2
    sy_j   = clip(softplus(sample_y_j . scale), 1e-10, 1e4);  v_j = sy_j^-1/2
    res    = 1 / (1 + u_i v_j d[i,j])
    out    = res * sigmoid(phi * (res - clip(cutoff, 0, 1000)))

Sharding: 2D grid over 8 cores, 4 x-blocks (NS=2048) x 2 y-blocks (MS=2048).

Device program per core (tile-granular PSUM dep tracking: all writes of a
psum tile are emitted before its single wide reader):
    denom*(1/C) = 1/C + (1/(256 C)) * T_ij
    T = 256*[ u sqx v + u v sqy - 2 u v dot ] accumulated per [128,2048]
    psum tile as 8 fp8-DoubleRow passes (x8 = 16*u_i*x, y8 = -32*v_j*y)
    plus one K=6 bf16 extension pass per 512-quarter carrying the
    u*sqx (x) 256v  and  u (x) 256*v*sqy  rank-1 terms (hi/lo split).
    out = Reciprocal(T/(256 C) + 1/C) on ACT (immediate scale+bias,
    one 2048-wide activation per tile), written bf16.
    Mask sigmoid folded into C (runtime-verified const fit; linear
    fallback multiplies (res + c0)*res on DVE).
Warmup matmuls ramp the PE clock while input DMA is in flight; inputs
are chunked/need-ordered across the two HWDGE queues.
"""

import os
import sys

sys.path.insert(0, "/opt/trn_rl_repo")

import numpy as np

N, M, D, S = 8192, 4096, 512, 16
XB, YB = 4, 2  # core grid
CORES = XB * YB
NS = N // XB  # 2048 rows of x per core
MS = M // YB  # 2048 cols (y rows) per core
PO = NS // 128  # 16 i-tiles
KT = D // 128  # 4 k-tiles
JT = MS // 512  # 4 j-quadrants per psum tile

SOFTPLUS_MIN = 1e-10
SOFTPLUS_MAX = 10000.0

GX = 16.0       # fp8 scale for x rows
GY = -32.0      # fp8 scale for y rows (carries the -2)
PSC = 256.0     # resulting P scale: P = 256 * (-2 u v dot)

_CACHE = {}

N_WARMUP = int(os.environ.get("N_WARMUP", "6"))


def _act_recip(nc, out, in_, scale, bias):
    """out = 1/(scale*in + bias) on the ACT engine (immediate scale/bias)."""
    import concourse.mybir as mybir

    eng = nc.scalar
    inputs = [eng.lower_ap(in_)]
    for val in (bias, scale, 0.0):
        inputs.append(
            mybir.ImmediateValue(dtype=mybir.dt.float32, value=float(val))
        )
    return eng.add_instruction(
        mybir.InstActivation(
            name=nc.get_next_instruction_name(),
            func=mybir.ActivationFunctionType.Reciprocal,
            ins=inputs,
            outs=[eng.lower_ap(out)],
        )
    )


def _fit_mask(phi_val, cutoff_val, R):
    """Linear + constant fits of sigmoid(phi*(t-c)) on [0,R]."""
    t = (np.cos(np.linspace(0, np.pi, 2001)) + 1) * (R / 2)
    g = 1.0 / (1.0 + np.exp(-phi_val * (t - cutoff_val)))
    m1_, m0_ = np.polyfit(t, g, 1)
    gerr = np.abs(np.polyval([m1_, m0_], t) - g) / np.abs(g)
    gmin, gmax = g.min(), g.max()
    c_const = 2.0 * gmin * gmax / (gmin + gmax)
    const_err = (gmax - gmin) / (gmax + gmin)
    return float(m0_), float(m1_), float(gerr.max()), float(c_const), float(const_err)


def _build(const_mask: bool, c0: float, bias: float, scale: float):
    import concourse.mybir as mybir
    import concourse.tile as tile
    from concourse import bacc

    dt = mybir.dt
    OP = mybir.AluOpType
    PM = mybir.MatmulPerfMode

    nc = bacc.Bacc("TRN2", target_bir_lowering=False)

    # chunked input layouts: per partition [kt2][grp][ktpair][512]
    x8_d = nc.dram_tensor("x8T_shard", [128, KT * NS], dt.float8e4,
                          kind="ExternalInput")
    y8_d = nc.dram_tensor("y8T_shard", [128, KT * MS], dt.float8e4,
                          kind="ExternalInput")
    extl_d = nc.dram_tensor("extL_shard", [6, NS], dt.bfloat16,
                            kind="ExternalInput")
    extr_d = nc.dram_tensor("extR_shard", [6, MS], dt.bfloat16,
                            kind="ExternalInput")
    out_d = nc.dram_tensor("out_shard", [NS, MS], dt.bfloat16,
                           kind="ExternalOutput")

    x8_v = x8_d.rearrange("p (kt2 g ktp i) -> p kt2 g ktp i", kt2=2, g=4, ktp=2)
    y8_v = y8_d.rearrange("p (kt2 g ktp j) -> p kt2 g ktp j", kt2=2, g=4, ktp=2)
    out_v = out_d.rearrange("(po pi) j -> pi po j", pi=128)

    with tile.TileContext(nc) as tc:
        with (
            tc.tile_pool(name="persist", bufs=1) as persist,
            tc.tile_pool(name="psum", bufs=2, space="PSUM") as psum_p,
            tc.tile_pool(name="stage", bufs=3) as stage,
        ):
            x8_sb = persist.tile([128, 2, 4, 2, 512], dt.float8e4)
            y8_sb = persist.tile([128, 2, 4, 2, 512], dt.float8e4)
            extl_sb = persist.tile([6, NS], dt.bfloat16)
            extr_sb = persist.tile([6, MS], dt.bfloat16)
            # warmup scratch for p-state ramp (vector is idle early)
            warm_sb = persist.tile([128, 2, 256], dt.float8e4)
            nc.vector.memset(warm_sb[:], 0.0)

            # input DMA, need-ordered on the two HWDGE queues (sync is
            # otherwise idle until the first output at ~12us)
            nc.sync.dma_start(extl_sb[:], extl_d[:, :])
            nc.sync.dma_start(extr_sb[:], extr_d[:, :])
            nc.sync.dma_start(y8_sb[:, 0], y8_v[:, 0])
            nc.sync.dma_start(y8_sb[:, 1], y8_v[:, 1])
            nc.scalar.dma_start(x8_sb[:, :, 0], x8_v[:, :, 0])
            for g in range(1, 4):
                nc.scalar.dma_start(x8_sb[:, :, g], x8_v[:, :, g])

            # warmup matmuls: no data deps; ramp the PE clock during DMA
            for w in range(N_WARMUP):
                wps = psum_p.tile([128, 2048], dt.float32, tag="mm",
                                  name=f"warm{w}")
                nc.tensor.matmul(
                    wps[:, 0:256],
                    lhsT=warm_sb[:, :, 0:128],
                    rhs=warm_sb[:, :, :],
                    start=True, stop=True,
                    perf_mode=PM.DoubleRow,
                )

            for po in range(PO):
                pog, pi = divmod(po, 4)
                lhsT = x8_sb[:, :, pog, :, pi * 128:(pi + 1) * 128]
                pst = psum_p.tile([128, 2048], dt.float32, tag="mm",
                                  name=f"mm{po}")
                ot = stage.tile([128, MS], dt.bfloat16, tag="ot",
                                name=f"ot{po}")
                if not const_mask:
                    rt = stage.tile([128, MS], dt.float32, tag="rt",
                                    name=f"rt{po}")
                last = po == PO - 1

                # all fp8-DR passes first, then all bf16 ext passes: the
                # dtype/mode switch breaks PE weight-load overlap, so only
                # switch once per tile
                for kt2 in range(2):
                    for jt in range(JT):
                        sl = slice(jt * 512, (jt + 1) * 512)
                        nc.tensor.matmul(
                            pst[:, sl],
                            lhsT=lhsT[:, kt2],
                            rhs=y8_sb[:, kt2, jt],
                            start=(kt2 == 0), stop=False,
                            perf_mode=PM.DoubleRow,
                        )
                for jt in range(JT):
                    sl = slice(jt * 512, (jt + 1) * 512)
                    nc.tensor.matmul(
                        pst[:, sl],
                        lhsT=extl_sb[:, po * 128:(po + 1) * 128],
                        rhs=extr_sb[:, sl],
                        start=False, stop=True,
                    )

                nh = 4 if last else 1
                w = MS // nh
                for h in range(nh):
                    d = slice(h * w, (h + 1) * w)
                    if const_mask:
                        _act_recip(nc, ot[:, d], pst[:, d], scale, bias)
                    else:
                        _act_recip(nc, rt[:, d], pst[:, d], scale, bias)
                        nc.vector.scalar_tensor_tensor(
                            ot[:, d], rt[:, d], c0, rt[:, d],
                            OP.add, OP.mult,
                        )
                    nc.sync.dma_start(out_v[:, po, d], ot[:, d])

    nc.compile()
    return nc


def _hi_lo(vec, bf16):
    hi = vec.astype(bf16)
    lo = (vec - hi.astype(np.float64)).astype(bf16)
    return hi, lo


def kernel(x, y, sample_x, sample_y, scale, cutoff, phi):
    import ml_dtypes
    from concourse.bass_utils import run_bass_kernel_spmd

    bf16 = ml_dtypes.bfloat16
    fp8 = ml_dtypes.float8_e4m3

    phi_val = float(np.asarray(phi).reshape(-1)[0])
    cutoff_val = float(np.clip(np.asarray(cutoff).reshape(-1)[0], 0.0, 1000.0))

    x64 = np.asarray(x, dtype=np.float32).astype(np.float64)
    y64 = np.asarray(y, dtype=np.float32).astype(np.float64)
    sc64 = np.asarray(scale, dtype=np.float32).astype(np.float64).reshape(-1)
    sx64 = np.asarray(sample_x, dtype=np.float32).astype(np.float64)
    sy64 = np.asarray(sample_y, dtype=np.float32).astype(np.float64)

    u = np.clip(np.log1p(np.exp(sx64 @ sc64)), SOFTPLUS_MIN, SOFTPLUS_MAX) ** -0.5
    v = np.clip(np.log1p(np.exp(sy64 @ sc64)), SOFTPLUS_MIN, SOFTPLUS_MAX) ** -0.5
    sqx = (x64 * x64).sum(axis=1)  # [N]
    sqy = (y64 * y64).sum(axis=1)  # [M]

    # res range from a subsample -> mask fit interval
    rng = np.random.default_rng(12345)
    ii = rng.integers(0, N, 4096)
    jj = rng.integers(0, M, 4096)
    dd = sqx[ii] + sqy[jj] - 2.0 * np.einsum("nd,nd->n", x64[ii], y64[jj])
    res_s = 1.0 / (1.0 + dd * (u[ii] * v[jj]))
    R = float(min(1.0, max(2.0 * res_s.max(), 0.01)))

    m0, m1, gerr, c_const, const_err = _fit_mask(phi_val, cutoff_val, R)
    const_mask = const_err < 6e-3
    if const_mask:
        cc = c_const
        c0 = 0.0
    else:
        assert gerr < 2e-3, f"mask linearization too coarse: {gerr}"
        cc = float(np.sqrt(m1))  # ACT produces sqrt(m1)*res; DVE applies mask
        c0 = m0 / cc
    bias = 1.0 / cc
    act_scale = 1.0 / (PSC * cc)

    key = (const_mask, round(c0, 9), round(bias, 9))
    if key not in _CACHE:
        _CACHE[key] = _build(const_mask, c0, bias, act_scale)
    nc = _CACHE[key]

    x8T = (GX * (u[:, None] * x64)).T.astype(fp8)   # [D, N]
    y8T = (GY * (v[:, None] * y64)).T.astype(fp8)   # [D, M]
    wv = PSC * v              # [M]
    brw = PSC * v * sqy       # [M]
    ah, al = _hi_lo(u * sqx, bf16)                  # [N]
    uh, ul = _hi_lo(u, bf16)
    wvh, wvl = _hi_lo(wv, bf16)
    bh, bl = _hi_lo(brw, bf16)
    extL = np.stack([ah, ah, al, uh, uh, ul])
    extR = np.stack([wvh, wvl, wvh, bh, bl, bh])

    in_maps = []
    for c in range(CORES):
        cx, cy = divmod(c, YB)
        si, sj = cx * NS, cy * MS
        xt = x8T[:, si:si + NS].reshape(2, 2, 128, 4, 512)  # [kt2,ktp,p,g,c]
        yt = y8T[:, sj:sj + MS].reshape(2, 2, 128, 4, 512)
        in_maps.append(
            {
                "x8T_shard": np.ascontiguousarray(
                    xt.transpose(2, 0, 3, 1, 4)).reshape(128, KT * NS),
                "y8T_shard": np.ascontiguousarray(
                    yt.transpose(2, 0, 3, 1, 4)).reshape(128, KT * MS),
                "extL_shard": np.ascontiguousarray(extL[:, si:si + NS]),
                "extR_shard": np.ascontiguousarray(extR[:, sj:sj + MS]),
            }
        )

    trace = bool(int(os.environ.get("KERNEL_TRACE", "0")))
    r = run_bass_kernel_spmd(nc, in_maps, core_ids=list(range(CORES)), trace=trace)
    kernel.last_results = r
    out = np.empty((N, M), dtype=np.float32)
    for c in range(CORES):
        cx, cy = divmod(c, YB)
        out[cx * NS:(cx + 1) * NS, cy * MS:(cy + 1) * MS] = np.asarray(
            r.results[c]["out_shard"]
        ).astype(np.float32)
    return out


if __name__ == "__main__":
    rng = np.random.default_rng(0)
    ins = {
        "x": rng.standard_normal((N, D), dtype=np.float32),
        "y": rng.standard_normal((M, D), dtype=np.float32),
        "sample_x": rng.random((N, S), dtype=np.float32),
        "sample_y": rng.random((M, S), dtype=np.float32),
        "scale": rng.random((S,), dtype=np.float32),
        "cutoff": np.full((1,), 0.1, dtype=np.float32),
        "phi": np.ones((1,), dtype=np.float32),
    }
    o = kernel(**ins)
    print(o.shape, o.dtype, o[:2, :4])


# revision 28
# speedup vs baseline: 1.6026x; 1.0054x over previous
"""Trainium2 Bass kernel for nn_BatchedCauchyKernel (v5c).

Computes, for x[N,D], y[M,D], sample_x[N,S], sample_y[M,S], scale[S]:
    d[i,j] = |x_i|^2 + |y_j|^2 - 2 x_i.y_j
    sx_i   = clip(softplus(sample_x_i . scale), 1e-10, 1e4);  u_i = sx_i^-1/2
    sy_j   = clip(softplus(sample_y_j . scale), 1e-10, 1e4);  v_j = sy_j^-1/2
    res    = 1 / (1 + u_i v_j d[i,j])
    out    = res * sigmoid(phi * (res - clip(cutoff, 0, 1000)))

Sharding: 2D grid over 8 cores, 4 x-blocks (NS=2048) x 2 y-blocks (MS=2048).

Device program per core (tile-granular PSUM dep tracking: all writes of a
psum tile are emitted before its single wide reader):
    denom*(1/C) = 1/C + (1/(256 C)) * T_ij
    T = 256*[ u sqx v + u v sqy - 2 u v dot ] accumulated per [128,2048]
    psum tile as 8 fp8-DoubleRow passes (x8 = 16*u_i*x, y8 = -32*v_j*y)
    plus one K=6 bf16 extension pass per 512-quarter carrying the
    u*sqx (x) 256v  and  u (x) 256*v*sqy  rank-1 terms (hi/lo split).
    out = Reciprocal(T/(256 C) + 1/C) on ACT (immediate scale+bias,
    one 2048-wide activation per tile), written bf16.
    Mask sigmoid folded into C (runtime-verified const fit; linear
    fallback multiplies (res + c0)*res on DVE).
Warmup matmuls ramp the PE clock while input DMA is in flight; inputs
are chunked/need-ordered across the two HWDGE queues.
"""

import os
import sys

sys.path.insert(0, "/opt/trn_rl_repo")

import numpy as np

N, M, D, S = 8192, 4096, 512, 16
XB, YB = 4, 2  # core grid
CORES = XB * YB
NS = N // XB  # 2048 rows of x per core
MS = M // YB  # 2048 cols (y rows) per core
PO = NS // 128  # 16 i-tiles
KT = D // 128  # 4 k-tiles
JT = MS // 512  # 4 j-quadrants per psum tile

SOFTPLUS_MIN = 1e-10
SOFTPLUS_MAX = 10000.0

GX = 16.0       # fp8 scale for x rows
GY = -32.0      # fp8 scale for y rows (carries the -2)
PSC = 256.0     # resulting P scale: P = 256 * (-2 u v dot)

_CACHE = {}

N_WARMUP = int(os.environ.get("N_WARMUP", "12"))


def _act_recip(nc, out, in_, scale, bias):
    """out = 1/(scale*in + bias) on the ACT engine (immediate scale/bias)."""
    import concourse.mybir as mybir

    eng = nc.scalar
    inputs = [eng.lower_ap(in_)]
    for val in (bias, scale, 0.0):
        inputs.append(
            mybir.ImmediateValue(dtype=mybir.dt.float32, value=float(val))
        )
    return eng.add_instruction(
        mybir.InstActivation(
            name=nc.get_next_instruction_name(),
            func=mybir.ActivationFunctionType.Reciprocal,
            ins=inputs,
            outs=[eng.lower_ap(out)],
        )
    )


def _fit_mask(phi_val, cutoff_val, R):
    """Linear + constant fits of sigmoid(phi*(t-c)) on [0,R]."""
    t = (np.cos(np.linspace(0, np.pi, 2001)) + 1) * (R / 2)
    g = 1.0 / (1.0 + np.exp(-phi_val * (t - cutoff_val)))
    m1_, m0_ = np.polyfit(t, g, 1)
    gerr = np.abs(np.polyval([m1_, m0_], t) - g) / np.abs(g)
    gmin, gmax = g.min(), g.max()
    c_const = 2.0 * gmin * gmax / (gmin + gmax)
    const_err = (gmax - gmin) / (gmax + gmin)
    return float(m0_), float(m1_), float(gerr.max()), float(c_const), float(const_err)


def _build(const_mask: bool, c0: float, bias: float, scale: float):
    import concourse.mybir as mybir
    import concourse.tile as tile
    from concourse import bacc

    dt = mybir.dt
    OP = mybir.AluOpType
    PM = mybir.MatmulPerfMode

    nc = bacc.Bacc("TRN2", target_bir_lowering=False)

    # chunked input layouts: per partition [kt2][grp][ktpair][512]
    x8_d = nc.dram_tensor("x8T_shard", [128, KT * NS], dt.float8e4,
                          kind="ExternalInput")
    y8_d = nc.dram_tensor("y8T_shard", [128, KT * MS], dt.float8e4,
                          kind="ExternalInput")
    extl_d = nc.dram_tensor("extL_shard", [6, NS], dt.bfloat16,
                            kind="ExternalInput")
    extr_d = nc.dram_tensor("extR_shard", [6, MS], dt.bfloat16,
                            kind="ExternalInput")
    out_d = nc.dram_tensor("out_shard", [NS, MS], dt.bfloat16,
                           kind="ExternalOutput")

    x8_v = x8_d.rearrange("p (kt2 g ktp i) -> p kt2 g ktp i", kt2=2, g=4, ktp=2)
    y8_v = y8_d.rearrange("p (kt2 g ktp j) -> p kt2 g ktp j", kt2=2, g=4, ktp=2)
    out_v = out_d.rearrange("(po pi) j -> pi po j", pi=128)

    with tile.TileContext(nc) as tc:
        with (
            tc.tile_pool(name="persist", bufs=1) as persist,
            tc.tile_pool(name="psum", bufs=2, space="PSUM") as psum_p,
            tc.tile_pool(name="stage", bufs=3) as stage,
        ):
            x8_sb = persist.tile([128, 2, 4, 2, 512], dt.float8e4)
            y8_sb = persist.tile([128, 2, 4, 2, 512], dt.float8e4)
            extl_sb = persist.tile([6, NS], dt.bfloat16)
            extr_sb = persist.tile([6, MS], dt.bfloat16)
            # warmup scratch for p-state ramp (vector is idle early)
            warm_sb = persist.tile([128, 2, 256], dt.float8e4)
            nc.vector.memset(warm_sb[:], 0.0)

            # input DMA, need-ordered on the two HWDGE queues (sync is
            # otherwise idle until the first output at ~12us); first-needed
            # chunks are small so the first matmul can start early
            for g in range(4):
                nc.sync.dma_start(y8_sb[:, 0, g], y8_v[:, 0, g])
            nc.sync.dma_start(y8_sb[:, 1], y8_v[:, 1])
            nc.scalar.dma_start(x8_sb[:, 0, 0], x8_v[:, 0, 0])
            nc.scalar.dma_start(x8_sb[:, 1, 0], x8_v[:, 1, 0])
            for g in range(1, 4):
                nc.scalar.dma_start(x8_sb[:, :, g], x8_v[:, :, g])
            nc.gpsimd.dma_start(extl_sb[:], extl_d[:, :])
            nc.gpsimd.dma_start(extr_sb[:], extr_d[:, :])

            # warmup matmuls: no data deps; ramp the PE clock during DMA
            for w in range(N_WARMUP):
                wps = psum_p.tile([128, 2048], dt.float32, tag="mm",
                                  name=f"warm{w}")
                nc.tensor.matmul(
                    wps[:, 0:256],
                    lhsT=warm_sb[:, :, 0:128],
                    rhs=warm_sb[:, :, :],
                    start=True, stop=True,
                    perf_mode=PM.DoubleRow,
                )

            ot2 = None
            for po in range(PO):
                pog, pi = divmod(po, 4)
                lhsT = x8_sb[:, :, pog, :, pi * 128:(pi + 1) * 128]
                pst = psum_p.tile([128, 2048], dt.float32, tag="mm",
                                  name=f"mm{po}")
                if po % 2 == 0:
                    ot2 = stage.tile([128, 2, MS], dt.bfloat16, tag="ot",
                                     name=f"ot{po}")
                ot = ot2[:, po % 2]
                if not const_mask:
                    rt = stage.tile([128, MS], dt.float32, tag="rt",
                                    name=f"rt{po}")
                last = po == PO - 1

                # all fp8-DR passes first, then all bf16 ext passes: the
                # dtype/mode switch breaks PE weight-load overlap, so only
                # switch once per tile
                for kt2 in range(2):
                    for jt in range(JT):
                        sl = slice(jt * 512, (jt + 1) * 512)
                        nc.tensor.matmul(
                            pst[:, sl],
                            lhsT=lhsT[:, kt2],
                            rhs=y8_sb[:, kt2, jt],
                            start=(kt2 == 0), stop=False,
                            perf_mode=PM.DoubleRow,
                        )
                for jt in range(JT):
                    sl = slice(jt * 512, (jt + 1) * 512)
                    nc.tensor.matmul(
                        pst[:, sl],
                        lhsT=extl_sb[:, po * 128:(po + 1) * 128],
                        rhs=extr_sb[:, sl],
                        start=False, stop=True,
                    )

                nh = 4 if last else 1
                w = MS // nh
                for h in range(nh):
                    d = slice(h * w, (h + 1) * w)
                    if const_mask:
                        _act_recip(nc, ot[:, d], pst[:, d], scale, bias)
                    else:
                        _act_recip(nc, rt[:, d], pst[:, d], scale, bias)
                        nc.vector.scalar_tensor_tensor(
                            ot[:, d], rt[:, d], c0, rt[:, d],
                            OP.add, OP.mult,
                        )
                    if last:
                        nc.sync.dma_start(out_v[:, po, d], ot[:, d])
                # one output DMA per po-pair, except the finely-sliced tail
                if po == PO - 2:
                    nc.sync.dma_start(out_v[:, po, :], ot[:, :])
                elif po % 2 == 1 and not last:
                    nc.sync.dma_start(out_v[:, po - 1:po + 1, :], ot2[:, :])

    nc.compile()
    return nc


def _hi_lo(vec, bf16):
    hi = vec.astype(bf16)
    lo = (vec - hi.astype(np.float64)).astype(bf16)
    return hi, lo


def kernel(x, y, sample_x, sample_y, scale, cutoff, phi):
    import ml_dtypes
    from concourse.bass_utils import run_bass_kernel_spmd

    bf16 = ml_dtypes.bfloat16
    fp8 = ml_dtypes.float8_e4m3

    phi_val = float(np.asarray(phi).reshape(-1)[0])
    cutoff_val = float(np.clip(np.asarray(cutoff).reshape(-1)[0], 0.0, 1000.0))

    x64 = np.asarray(x, dtype=np.float32).astype(np.float64)
    y64 = np.asarray(y, dtype=np.float32).astype(np.float64)
    sc64 = np.asarray(scale, dtype=np.float32).astype(np.float64).reshape(-1)
    sx64 = np.asarray(sample_x, dtype=np.float32).astype(np.float64)
    sy64 = np.asarray(sample_y, dtype=np.float32).astype(np.float64)

    u = np.clip(np.log1p(np.exp(sx64 @ sc64)), SOFTPLUS_MIN, SOFTPLUS_MAX) ** -0.5
    v = np.clip(np.log1p(np.exp(sy64 @ sc64)), SOFTPLUS_MIN, SOFTPLUS_MAX) ** -0.5
    sqx = (x64 * x64).sum(axis=1)  # [N]
    sqy = (y64 * y64).sum(axis=1)  # [M]

    # res range from a subsample -> mask fit interval
    rng = np.random.default_rng(12345)
    ii = rng.integers(0, N, 4096)
    jj = rng.integers(0, M, 4096)
    dd = sqx[ii] + sqy[jj] - 2.0 * np.einsum("nd,nd->n", x64[ii], y64[jj])
    res_s = 1.0 / (1.0 + dd * (u[ii] * v[jj]))
    R = float(min(1.0, max(2.0 * res_s.max(), 0.01)))

    m0, m1, gerr, c_const, const_err = _fit_mask(phi_val, cutoff_val, R)
    const_mask = const_err < 6e-3
    if const_mask:
        cc = c_const
        c0 = 0.0
    else:
        assert gerr < 2e-3, f"mask linearization too coarse: {gerr}"
        cc = float(np.sqrt(m1))  # ACT produces sqrt(m1)*res; DVE applies mask
        c0 = m0 / cc
    bias = 1.0 / cc
    act_scale = 1.0 / (PSC * cc)

    key = (const_mask, round(c0, 9), round(bias, 9))
    if key not in _CACHE:
        _CACHE[key] = _build(const_mask, c0, bias, act_scale)
    nc = _CACHE[key]

    x8T = (GX * (u[:, None] * x64)).T.astype(fp8)   # [D, N]
    y8T = (GY * (v[:, None] * y64)).T.astype(fp8)   # [D, M]
    wv = PSC * v              # [M]
    brw = PSC * v * sqy       # [M]
    ah, al = _hi_lo(u * sqx, bf16)                  # [N]
    uh, ul = _hi_lo(u, bf16)
    wvh, wvl = _hi_lo(wv, bf16)
    bh, bl = _hi_lo(brw, bf16)
    extL = np.stack([ah, ah, al, uh, uh, ul])
    extR = np.stack([wvh, wvl, wvh, bh, bl, bh])

    in_maps = []
    for c in range(CORES):
        cx, cy = divmod(c, YB)
        si, sj = cx * NS, cy * MS
        xt = x8T[:, si:si + NS].reshape(2, 2, 128, 4, 512)  # [kt2,ktp,p,g,c]
        yt = y8T[:, sj:sj + MS].reshape(2, 2, 128, 4, 512)
        in_maps.append(
            {
                "x8T_shard": np.ascontiguousarray(
                    xt.transpose(2, 0, 3, 1, 4)).reshape(128, KT * NS),
                "y8T_shard": np.ascontiguousarray(
                    yt.transpose(2, 0, 3, 1, 4)).reshape(128, KT * MS),
                "extL_shard": np.ascontiguousarray(extL[:, si:si + NS]),
                "extR_shard": np.ascontiguousarray(extR[:, sj:sj + MS]),
            }
        )

    trace = bool(int(os.environ.get("KERNEL_TRACE", "0")))
    r = run_bass_kernel_spmd(nc, in_maps, core_ids=list(range(CORES)), trace=trace)
    kernel.last_results = r
    out = np.empty((N, M), dtype=np.float32)
    for c in range(CORES):
        cx, cy = divmod(c, YB)
        out[cx * NS:(cx + 1) * NS, cy * MS:(cy + 1) * MS] = np.asarray(
            r.results[c]["out_shard"]
        ).astype(np.float32)
    return out


if __name__ == "__main__":
    rng = np.random.default_rng(0)
    ins = {
        "x": rng.standard_normal((N, D), dtype=np.float32),
        "y": rng.standard_normal((M, D), dtype=np.float32),
        "sample_x": rng.random((N, S), dtype=np.float32),
        "sample_y": rng.random((M, S), dtype=np.float32),
        "scale": rng.random((S,), dtype=np.float32),
        "cutoff": np.full((1,), 0.1, dtype=np.float32),
        "phi": np.ones((1,), dtype=np.float32),
    }
    o = kernel(**ins)
    print(o.shape, o.dtype, o[:2, :4])


# revision 29
# speedup vs baseline: 1.7206x; 1.0736x over previous
"""Trainium2 Bass kernel for nn_BatchedCauchyKernel (v6).

Computes, for x[N,D], y[M,D], sample_x[N,S], sample_y[M,S], scale[S]:
    d[i,j] = |x_i|^2 + |y_j|^2 - 2 x_i.y_j
    sx_i   = clip(softplus(sample_x_i . scale), 1e-10, 1e4);  u_i = sx_i^-1/2
    sy_j   = clip(softplus(sample_y_j . scale), 1e-10, 1e4);  v_j = sy_j^-1/2
    res    = 1 / (1 + u_i v_j d[i,j])
    out    = res * sigmoid(phi * (res - clip(cutoff, 0, 1000)))

Sharding: 2D grid over 8 cores, 4 x-blocks (NS=2048) x 2 y-blocks (MS=2048).

Structure (hybrid injection; PE column-streaming costs 216ns per 512-col
pass regardless of K, PSUM is reachable only from PE/ACT/DVE, and PSUM
dependency tracking is tile-granular):
    denom/C = 1/C + (u_i/C) * [ v_j sqx_i + v_j sqy_j - 2 v_j dot_ij ]
  - per po two [128,1024] psum tiles; 2 fp8-DoubleRow passes per 512-col
    quarter accumulate P = 256*(-2 v_j dot) (x8 = 16*x, y8 = -32*v_j*y)
  - "pe" tile (quarters 0-1): + one K=5 bf16 extension pass per quarter
    (rows sqxh*wvh, sqxh*wvl, sqxl*wvh, 1*Bh, 1*Bl; wv = 256v, B = 256
    v*sqy), emitted after all DR passes (mode switches break weight-load
    overlap)
  - "dv" tile (quarters 2-3): one DVE tensor_tensor adds T1[po] =
    v_rep*sqx_po + B_rep, built by DVE scalar_tensor_tensor into a ring;
    v_rep/B_rep replicated by K=2 hi/lo PE-broadcast matmuls at startup
    (which double as clock warmup together with garbage warmup matmuls)
  - ACT Reciprocal with per-partition scale AP u_i/(256 C), immediate
    bias 1/C -> bf16 out; one 1024-wide activation per psum tile.
    Mask sigmoid folded into C (runtime-verified const fit; linear
    fallback applies (res + c0)*res on DVE).
"""

import os
import sys

sys.path.insert(0, "/opt/trn_rl_repo")

import numpy as np

N, M, D, S = 8192, 4096, 512, 16
XB, YB = 4, 2  # core grid
CORES = XB * YB
NS = N // XB  # 2048 rows of x per core
MS = M // YB  # 2048 cols (y rows) per core
PO = NS // 128  # 16 i-tiles
KT = D // 128  # 4 k-tiles
JT = MS // 512  # 4 j-quadrants per output row-block

SOFTPLUS_MIN = 1e-10
SOFTPLUS_MAX = 10000.0

GX = 16.0       # fp8 scale for x rows
GY = -32.0      # fp8 scale for y rows (carries the -2)
PSC = 256.0     # P = 256 * (-2 v dot)

_CACHE = {}

RING = int(os.environ.get("RING", "12"))
N_WARMUP = int(os.environ.get("N_WARMUP", "6"))


def _act_recip(nc, out, in_, scale, bias):
    """out = 1/(scale*in + bias); scale may be a [128,1] AP."""
    import concourse.mybir as mybir

    eng = nc.scalar
    inputs = [eng.lower_ap(in_)]
    inputs.append(mybir.ImmediateValue(dtype=mybir.dt.float32, value=float(bias)))
    if hasattr(scale, "space"):
        inputs.append(eng.lower_ap(scale))
    else:
        inputs.append(
            mybir.ImmediateValue(dtype=mybir.dt.float32, value=float(scale))
        )
    inputs.append(mybir.ImmediateValue(dtype=mybir.dt.float32, value=0.0))
    return eng.add_instruction(
        mybir.InstActivation(
            name=nc.get_next_instruction_name(),
            func=mybir.ActivationFunctionType.Reciprocal,
            ins=inputs,
            outs=[eng.lower_ap(out)],
        )
    )


def _fit_mask(phi_val, cutoff_val, R):
    """Linear + constant fits of sigmoid(phi*(t-c)) on [0,R]."""
    t = (np.cos(np.linspace(0, np.pi, 2001)) + 1) * (R / 2)
    g = 1.0 / (1.0 + np.exp(-phi_val * (t - cutoff_val)))
    m1_, m0_ = np.polyfit(t, g, 1)
    gerr = np.abs(np.polyval([m1_, m0_], t) - g) / np.abs(g)
    gmin, gmax = g.min(), g.max()
    c_const = 2.0 * gmin * gmax / (gmin + gmax)
    const_err = (gmax - gmin) / (gmax + gmin)
    return float(m0_), float(m1_), float(gerr.max()), float(c_const), float(const_err)


def _build(const_mask: bool, c0: float, bias: float):
    import concourse.mybir as mybir
    import concourse.tile as tile
    from concourse import bacc

    dt = mybir.dt
    OP = mybir.AluOpType
    PM = mybir.MatmulPerfMode

    nc = bacc.Bacc("TRN2", target_bir_lowering=False)

    x8_d = nc.dram_tensor("x8T_shard", [128, KT * NS], dt.float8e4,
                          kind="ExternalInput")
    y8_d = nc.dram_tensor("y8T_shard", [128, KT * MS], dt.float8e4,
                          kind="ExternalInput")
    sqx_d = nc.dram_tensor("sqx_shard", [128, PO], dt.float32,
                           kind="ExternalInput")
    uc_d = nc.dram_tensor("uc_shard", [128, PO], dt.float32,
                          kind="ExternalInput")
    extl_d = nc.dram_tensor("extL_shard", [5, NS], dt.bfloat16,
                            kind="ExternalInput")
    extr_d = nc.dram_tensor("extR_shard", [5, MS], dt.bfloat16,
                            kind="ExternalInput")
    vrow_d = nc.dram_tensor("vrow_shard", [2, MS], dt.bfloat16,
                            kind="ExternalInput")
    brow_d = nc.dram_tensor("brow_shard", [2, MS], dt.bfloat16,
                            kind="ExternalInput")
    ones_d = nc.dram_tensor("ones_row", [2, 128], dt.bfloat16,
                            kind="ExternalInput")
    out_d = nc.dram_tensor("out_shard", [NS, MS], dt.bfloat16,
                           kind="ExternalOutput")

    x8_v = x8_d.rearrange("p (kt2 g ktp i) -> p kt2 g ktp i", kt2=2, g=4, ktp=2)
    y8_v = y8_d.rearrange("p (kt2 g ktp j) -> p kt2 g ktp j", kt2=2, g=4, ktp=2)
    out_v = out_d.rearrange("(po pi) j -> pi po j", pi=128)

    with tile.TileContext(nc) as tc:
        with (
            tc.tile_pool(name="persist", bufs=1) as persist,
            tc.tile_pool(name="psum", bufs=2, space="PSUM") as psum_p,
            tc.tile_pool(name="stage", bufs=3) as stage,
        ):
            x8_sb = persist.tile([128, 2, 4, 2, 512], dt.float8e4)
            y8_sb = persist.tile([128, 2, 4, 2, 512], dt.float8e4)
            sqx_sb = persist.tile([128, PO], dt.float32)
            uc_sb = persist.tile([128, PO], dt.float32)
            extl_sb = persist.tile([5, NS], dt.bfloat16)
            extr_sb = persist.tile([5, MS], dt.bfloat16)
            vrow_sb = persist.tile([2, MS], dt.bfloat16)
            brow_sb = persist.tile([2, MS], dt.bfloat16)
            ones_sb = persist.tile([2, 128], dt.bfloat16)
            v_rep = persist.tile([128, 1024], dt.float32)
            B_rep = persist.tile([128, 1024], dt.float32)
            t1_sb = persist.tile([128, RING, 1024], dt.float32)
            warm_sb = persist.tile([128, 2, 256], dt.float8e4)
            nc.vector.memset(warm_sb[:], 0.0)

            # --- input DMA, need-ordered ---
            # scalar HWDGE: tiny rows for the broadcasts, then x (pog0 split)
            nc.scalar.dma_start(vrow_sb[:], vrow_d[:, :])
            nc.scalar.dma_start(brow_sb[:], brow_d[:, :])
            nc.scalar.dma_start(ones_sb[:], ones_d[:, :])
            nc.scalar.dma_start(x8_sb[:, 0, 0], x8_v[:, 0, 0])
            nc.scalar.dma_start(x8_sb[:, 1, 0], x8_v[:, 1, 0])
            for g in range(1, 4):
                nc.scalar.dma_start(x8_sb[:, :, g], x8_v[:, :, g])
            # sync HWDGE: y kt0 in jt chunks, then y kt1
            for g in range(4):
                nc.sync.dma_start(y8_sb[:, 0, g], y8_v[:, 0, g])
            nc.sync.dma_start(y8_sb[:, 1], y8_v[:, 1])
            # gpsimd SWDGE: small side tensors
            nc.gpsimd.dma_start(extl_sb[:], extl_d[:, :])
            nc.gpsimd.dma_start(extr_sb[:], extr_d[:, :])
            nc.gpsimd.dma_start(sqx_sb[:], sqx_d[:, :])
            nc.gpsimd.dma_start(uc_sb[:], uc_d[:, :])

            # --- warmup + j-row broadcasts (quarters 2-3 only) ---
            for w in range(N_WARMUP):
                wps = psum_p.tile([128, 1024], dt.float32, tag="pe",
                                  name=f"warm{w}")
                nc.tensor.matmul(
                    wps[:, 0:256],
                    lhsT=warm_sb[:, :, 0:128],
                    rhs=warm_sb[:, :, :],
                    start=True, stop=True,
                    perf_mode=PM.DoubleRow,
                )
            bcv = psum_p.tile([128, 1024], dt.float32, tag="dv", name="bcv")
            for q in range(2):
                nc.tensor.matmul(
                    bcv[:, q * 512:(q + 1) * 512],
                    lhsT=ones_sb[:, :],
                    rhs=vrow_sb[:, 1024 + q * 512:1024 + (q + 1) * 512],
                    start=True, stop=True,
                )
            bcb = psum_p.tile([128, 1024], dt.float32, tag="dv", name="bcb")
            for q in range(2):
                nc.tensor.matmul(
                    bcb[:, q * 512:(q + 1) * 512],
                    lhsT=ones_sb[:, :],
                    rhs=brow_sb[:, 1024 + q * 512:1024 + (q + 1) * 512],
                    start=True, stop=True,
                )
            nc.vector.tensor_copy(v_rep[:], bcv[:])
            nc.scalar.copy(B_rep[:], bcb[:])

            # --- T1[po] = v_rep * sqx_po + B_rep (DVE-only op, ring) ---
            def emit_t1(po):
                nc.vector.scalar_tensor_tensor(
                    t1_sb[:, po % RING, :],
                    v_rep[:], sqx_sb[:, po:po + 1], B_rep[:],
                    OP.mult, OP.add,
                )

            warm = min(3, PO)
            for po in range(warm):
                emit_t1(po)

            for po in range(PO):
                pog, pi = divmod(po, 4)
                lhsT = x8_sb[:, :, pog, :, pi * 128:(pi + 1) * 128]
                ot = stage.tile([128, MS], dt.bfloat16, tag="ot",
                                name=f"ot{po}")
                if not const_mask:
                    rt = stage.tile([128, MS], dt.float32, tag="rt",
                                    name=f"rt{po}")
                last = po == PO - 1
                uc = uc_sb[:, po:po + 1]

                def epi(dst0, src_t, width):
                    nhl = 2 if last else 1
                    wl = width // nhl
                    for h in range(nhl):
                        dsl = slice(dst0 + h * wl, dst0 + (h + 1) * wl)
                        ssl = slice(h * wl, (h + 1) * wl)
                        if const_mask:
                            _act_recip(nc, ot[:, dsl], src_t[:, ssl], uc, bias)
                        else:
                            _act_recip(nc, rt[:, dsl], src_t[:, ssl], uc, bias)
                            nc.vector.scalar_tensor_tensor(
                                ot[:, dsl], rt[:, dsl], c0, rt[:, dsl],
                                OP.add, OP.mult,
                            )
                        if last:
                            nc.sync.dma_start(out_v[:, po, dsl], ot[:, dsl])

                # pe tile: quarters 0-1 (DR passes then ext passes)
                pe_t = psum_p.tile([128, 1024], dt.float32, tag="pe",
                                   name=f"pe{po}")
                for kt2 in range(2):
                    for jt in range(2):
                        nc.tensor.matmul(
                            pe_t[:, jt * 512:(jt + 1) * 512],
                            lhsT=lhsT[:, kt2],
                            rhs=y8_sb[:, kt2, jt],
                            start=(kt2 == 0), stop=False,
                            perf_mode=PM.DoubleRow,
                        )
                for jt in range(2):
                    nc.tensor.matmul(
                        pe_t[:, jt * 512:(jt + 1) * 512],
                        lhsT=extl_sb[:, po * 128:(po + 1) * 128],
                        rhs=extr_sb[:, jt * 512:(jt + 1) * 512],
                        start=False, stop=True,
                    )
                epi(0, pe_t, 1024)

                # dv tile: quarters 2-3 (DR passes then DVE T1 add)
                dv_t = psum_p.tile([128, 1024], dt.float32, tag="dv",
                                   name=f"dv{po}")
                for kt2 in range(2):
                    for jt in range(2, 4):
                        nc.tensor.matmul(
                            dv_t[:, (jt - 2) * 512:(jt - 1) * 512],
                            lhsT=lhsT[:, kt2],
                            rhs=y8_sb[:, kt2, jt],
                            start=(kt2 == 0), stop=(kt2 == 1),
                            perf_mode=PM.DoubleRow,
                        )
                nc.vector.tensor_tensor(
                    dv_t[:], t1_sb[:, po % RING, :], dv_t[:], OP.add,
                )
                epi(1024, dv_t, 1024)

                if po + warm < PO:
                    emit_t1(po + warm)
                if not last:
                    nc.sync.dma_start(out_v[:, po, :], ot[:, :])

    nc.compile()
    return nc


def _hi_lo(vec, bf16):
    hi = vec.astype(bf16)
    lo = (vec - hi.astype(np.float64)).astype(bf16)
    return hi, lo


def kernel(x, y, sample_x, sample_y, scale, cutoff, phi):
    import ml_dtypes
    from concourse.bass_utils import run_bass_kernel_spmd

    bf16 = ml_dtypes.bfloat16
    fp8 = ml_dtypes.float8_e4m3

    phi_val = float(np.asarray(phi).reshape(-1)[0])
    cutoff_val = float(np.clip(np.asarray(cutoff).reshape(-1)[0], 0.0, 1000.0))

    x64 = np.asarray(x, dtype=np.float32).astype(np.float64)
    y64 = np.asarray(y, dtype=np.float32).astype(np.float64)
    sc64 = np.asarray(scale, dtype=np.float32).astype(np.float64).reshape(-1)
    sx64 = np.asarray(sample_x, dtype=np.float32).astype(np.float64)
    sy64 = np.asarray(sample_y, dtype=np.float32).astype(np.float64)

    u = np.clip(np.log1p(np.exp(sx64 @ sc64)), SOFTPLUS_MIN, SOFTPLUS_MAX) ** -0.5
    v = np.clip(np.log1p(np.exp(sy64 @ sc64)), SOFTPLUS_MIN, SOFTPLUS_MAX) ** -0.5
    sqx = (x64 * x64).sum(axis=1)  # [N]
    sqy = (y64 * y64).sum(axis=1)  # [M]

    # res range from a subsample -> mask fit interval
    rng = np.random.default_rng(12345)
    ii = rng.integers(0, N, 4096)
    jj = rng.integers(0, M, 4096)
    dd = sqx[ii] + sqy[jj] - 2.0 * np.einsum("nd,nd->n", x64[ii], y64[jj])
    res_s = 1.0 / (1.0 + dd * (u[ii] * v[jj]))
    R = float(min(1.0, max(2.0 * res_s.max(), 0.01)))

    m0, m1, gerr, c_const, const_err = _fit_mask(phi_val, cutoff_val, R)
    const_mask = const_err < 6e-3
    if const_mask:
        cc = c_const
        c0 = 0.0
    else:
        assert gerr < 2e-3, f"mask linearization too coarse: {gerr}"
        cc = float(np.sqrt(m1))
        c0 = m0 / cc
    bias = 1.0 / cc
    sc_p = u / (PSC * cc)   # per-partition ACT scale over N

    key = (const_mask, round(c0, 9), round(bias, 9), RING)
    if key not in _CACHE:
        _CACHE[key] = _build(const_mask, c0, bias)
    nc = _CACHE[key]

    x8T = (GX * x64).T.astype(fp8)                  # [D, N]
    y8T = (GY * (v[:, None] * y64)).T.astype(fp8)   # [D, M]
    wv = PSC * v              # [M]
    brw = PSC * v * sqy       # [M]
    sqxh, sqxl = _hi_lo(sqx, bf16)
    wvh, wvl = _hi_lo(wv, bf16)
    bh, bl = _hi_lo(brw, bf16)
    extL = np.stack([sqxh, sqxh, sqxl,
                     np.ones(N, dtype=bf16), np.ones(N, dtype=bf16)])
    extR = np.stack([wvh, wvl, wvh, bh, bl])
    ones_row = np.ones((2, 128), dtype=bf16)

    in_maps = []
    for c in range(CORES):
        cx, cy = divmod(c, YB)
        si, sj = cx * NS, cy * MS
        xt = x8T[:, si:si + NS].reshape(2, 2, 128, 4, 512)  # [kt2,ktp,p,g,c]
        yt = y8T[:, sj:sj + MS].reshape(2, 2, 128, 4, 512)
        in_maps.append(
            {
                "x8T_shard": np.ascontiguousarray(
                    xt.transpose(2, 0, 3, 1, 4)).reshape(128, KT * NS),
                "y8T_shard": np.ascontiguousarray(
                    yt.transpose(2, 0, 3, 1, 4)).reshape(128, KT * MS),
                "sqx_shard": np.ascontiguousarray(
                    sqx[si:si + NS].reshape(PO, 128).T.astype(np.float32)),
                "uc_shard": np.ascontiguousarray(
                    sc_p[si:si + NS].reshape(PO, 128).T.astype(np.float32)),
                "extL_shard": np.ascontiguousarray(extL[:, si:si + NS]),
                "extR_shard": np.ascontiguousarray(extR[:, sj:sj + MS]),
                "vrow_shard": np.ascontiguousarray(
                    np.stack(_hi_lo(wv[sj:sj + MS], bf16))),
                "brow_shard": np.ascontiguousarray(
                    np.stack(_hi_lo(brw[sj:sj + MS], bf16))),
                "ones_row": ones_row,
            }
        )

    trace = bool(int(os.environ.get("KERNEL_TRACE", "0")))
    r = run_bass_kernel_spmd(nc, in_maps, core_ids=list(range(CORES)), trace=trace)
    kernel.last_results = r
    out = np.empty((N, M), dtype=np.float32)
    for c in range(CORES):
        cx, cy = divmod(c, YB)
        out[cx * NS:(cx + 1) * NS, cy * MS:(cy + 1) * MS] = np.asarray(
            r.results[c]["out_shard"]
        ).astype(np.float32)
    return out


if __name__ == "__main__":
    rng = np.random.default_rng(0)
    ins = {
        "x": rng.standard_normal((N, D), dtype=np.float32),
        "y": rng.standard_normal((M, D), dtype=np.float32),
        "sample_x": rng.random((N, S), dtype=np.float32),
        "sample_y": rng.random((M, S), dtype=np.float32),
        "scale": rng.random((S,), dtype=np.float32),
        "cutoff": np.full((1,), 0.1, dtype=np.float32),
        "phi": np.ones((1,), dtype=np.float32),
    }
    o = kernel(**ins)
    print(o.shape, o.dtype, o[:2, :4])


# revision 30
# speedup vs baseline: 1.7372x; 1.0096x over previous
"""Trainium2 Bass kernel for nn_BatchedCauchyKernel (v6).

Computes, for x[N,D], y[M,D], sample_x[N,S], sample_y[M,S], scale[S]:
    d[i,j] = |x_i|^2 + |y_j|^2 - 2 x_i.y_j
    sx_i   = clip(softplus(sample_x_i . scale), 1e-10, 1e4);  u_i = sx_i^-1/2
    sy_j   = clip(softplus(sample_y_j . scale), 1e-10, 1e4);  v_j = sy_j^-1/2
    res    = 1 / (1 + u_i v_j d[i,j])
    out    = res * sigmoid(phi * (res - clip(cutoff, 0, 1000)))

Sharding: 2D grid over 8 cores, 4 x-blocks (NS=2048) x 2 y-blocks (MS=2048).

Structure (hybrid injection; PE column-streaming costs 216ns per 512-col
pass regardless of K, PSUM is reachable only from PE/ACT/DVE, and PSUM
dependency tracking is tile-granular):
    denom/C = 1/C + (u_i/C) * [ v_j sqx_i + v_j sqy_j - 2 v_j dot_ij ]
  - per po two [128,1024] psum tiles; 2 fp8-DoubleRow passes per 512-col
    quarter accumulate P = 256*(-2 v_j dot) (x8 = 16*x, y8 = -32*v_j*y)
  - "pe" tile (quarters 0-1): + one K=5 bf16 extension pass per quarter
    (rows sqxh*wvh, sqxh*wvl, sqxl*wvh, 1*Bh, 1*Bl; wv = 256v, B = 256
    v*sqy), emitted after all DR passes (mode switches break weight-load
    overlap)
  - "dv" tile (quarters 2-3): one DVE tensor_tensor adds T1[po] =
    v_rep*sqx_po + B_rep, built by DVE scalar_tensor_tensor into a ring;
    v_rep/B_rep replicated by K=2 hi/lo PE-broadcast matmuls at startup
    (which double as clock warmup together with garbage warmup matmuls)
  - ACT Reciprocal with per-partition scale AP u_i/(256 C), immediate
    bias 1/C -> bf16 out; one 1024-wide activation per psum tile.
    Mask sigmoid folded into C (runtime-verified const fit; linear
    fallback applies (res + c0)*res on DVE).
"""

import os
import sys

sys.path.insert(0, "/opt/trn_rl_repo")

import numpy as np

N, M, D, S = 8192, 4096, 512, 16
XB, YB = 4, 2  # core grid
CORES = XB * YB
NS = N // XB  # 2048 rows of x per core
MS = M // YB  # 2048 cols (y rows) per core
PO = NS // 128  # 16 i-tiles
KT = D // 128  # 4 k-tiles
JT = MS // 512  # 4 j-quadrants per output row-block

SOFTPLUS_MIN = 1e-10
SOFTPLUS_MAX = 10000.0

GX = 16.0       # fp8 scale for x rows
GY = -32.0      # fp8 scale for y rows (carries the -2)
PSC = 256.0     # P = 256 * (-2 v dot)

_CACHE = {}

RING = int(os.environ.get("RING", "12"))
N_WARMUP = int(os.environ.get("N_WARMUP", "6"))


def _act_recip(nc, out, in_, scale, bias):
    """out = 1/(scale*in + bias); scale may be a [128,1] AP."""
    import concourse.mybir as mybir

    eng = nc.scalar
    inputs = [eng.lower_ap(in_)]
    inputs.append(mybir.ImmediateValue(dtype=mybir.dt.float32, value=float(bias)))
    if hasattr(scale, "space"):
        inputs.append(eng.lower_ap(scale))
    else:
        inputs.append(
            mybir.ImmediateValue(dtype=mybir.dt.float32, value=float(scale))
        )
    inputs.append(mybir.ImmediateValue(dtype=mybir.dt.float32, value=0.0))
    return eng.add_instruction(
        mybir.InstActivation(
            name=nc.get_next_instruction_name(),
            func=mybir.ActivationFunctionType.Reciprocal,
            ins=inputs,
            outs=[eng.lower_ap(out)],
        )
    )


def _fit_mask(phi_val, cutoff_val, R):
    """Linear + constant fits of sigmoid(phi*(t-c)) on [0,R]."""
    t = (np.cos(np.linspace(0, np.pi, 2001)) + 1) * (R / 2)
    g = 1.0 / (1.0 + np.exp(-phi_val * (t - cutoff_val)))
    m1_, m0_ = np.polyfit(t, g, 1)
    gerr = np.abs(np.polyval([m1_, m0_], t) - g) / np.abs(g)
    gmin, gmax = g.min(), g.max()
    c_const = 2.0 * gmin * gmax / (gmin + gmax)
    const_err = (gmax - gmin) / (gmax + gmin)
    return float(m0_), float(m1_), float(gerr.max()), float(c_const), float(const_err)


def _build(const_mask: bool, c0: float, bias: float):
    import concourse.mybir as mybir
    import concourse.tile as tile
    from concourse import bacc

    dt = mybir.dt
    OP = mybir.AluOpType
    PM = mybir.MatmulPerfMode

    nc = bacc.Bacc("TRN2", target_bir_lowering=False)

    x8_d = nc.dram_tensor("x8T_shard", [128, KT * NS], dt.float8e4,
                          kind="ExternalInput")
    y8_d = nc.dram_tensor("y8T_shard", [128, KT * MS], dt.float8e4,
                          kind="ExternalInput")
    sqx_d = nc.dram_tensor("sqx_shard", [128, PO], dt.float32,
                           kind="ExternalInput")
    uc_d = nc.dram_tensor("uc_shard", [128, PO], dt.float32,
                          kind="ExternalInput")
    extl_d = nc.dram_tensor("extL_shard", [5, NS], dt.bfloat16,
                            kind="ExternalInput")
    extr_d = nc.dram_tensor("extR_shard", [5, MS], dt.bfloat16,
                            kind="ExternalInput")
    vrow_d = nc.dram_tensor("vrow_shard", [2, MS], dt.bfloat16,
                            kind="ExternalInput")
    brow_d = nc.dram_tensor("brow_shard", [2, MS], dt.bfloat16,
                            kind="ExternalInput")
    ones_d = nc.dram_tensor("ones_row", [2, 128], dt.bfloat16,
                            kind="ExternalInput")
    out_d = nc.dram_tensor("out_shard", [NS, MS], dt.bfloat16,
                           kind="ExternalOutput")

    x8_v = x8_d.rearrange("p (kt2 g ktp i) -> p kt2 g ktp i", kt2=2, g=4, ktp=2)
    y8_v = y8_d.rearrange("p (kt2 g ktp j) -> p kt2 g ktp j", kt2=2, g=4, ktp=2)
    out_v = out_d.rearrange("(po pi) j -> pi po j", pi=128)

    with tile.TileContext(nc) as tc:
        with (
            tc.tile_pool(name="persist", bufs=1) as persist,
            tc.tile_pool(name="psum", bufs=2, space="PSUM") as psum_p,
            tc.tile_pool(name="stage", bufs=3) as stage,
        ):
            x8_sb = persist.tile([128, 2, 4, 2, 512], dt.float8e4)
            y8_sb = persist.tile([128, 2, 4, 2, 512], dt.float8e4)
            sqx_sb = persist.tile([128, PO], dt.float32)
            uc_sb = persist.tile([128, PO], dt.float32)
            extl_sb = persist.tile([5, NS], dt.bfloat16)
            extr_sb = persist.tile([5, MS], dt.bfloat16)
            vrow_sb = persist.tile([2, MS], dt.bfloat16)
            brow_sb = persist.tile([2, MS], dt.bfloat16)
            ones_sb = persist.tile([2, 128], dt.bfloat16)
            v_rep = persist.tile([128, 1024], dt.float32)
            B_rep = persist.tile([128, 1024], dt.float32)
            t1_sb = persist.tile([128, RING, 1024], dt.float32)
            warm_sb = persist.tile([128, 2, 256], dt.float8e4)
            nc.vector.memset(warm_sb[:], 0.0)

            # --- input DMA, need-ordered ---
            # scalar HWDGE: tiny rows for the broadcasts, then x (pog0 split)
            nc.scalar.dma_start(vrow_sb[:], vrow_d[:, :])
            nc.scalar.dma_start(brow_sb[:], brow_d[:, :])
            nc.scalar.dma_start(ones_sb[:], ones_d[:, :])
            nc.scalar.dma_start(x8_sb[:, 0, 0], x8_v[:, 0, 0])
            nc.scalar.dma_start(x8_sb[:, 1, 0], x8_v[:, 1, 0])
            for g in range(1, 4):
                nc.scalar.dma_start(x8_sb[:, :, g], x8_v[:, :, g])
            # sync HWDGE: y kt0 in jt chunks, then y kt1
            for g in range(4):
                nc.sync.dma_start(y8_sb[:, 0, g], y8_v[:, 0, g])
            nc.sync.dma_start(y8_sb[:, 1], y8_v[:, 1])
            # gpsimd SWDGE: small side tensors
            nc.gpsimd.dma_start(extl_sb[:], extl_d[:, :])
            nc.gpsimd.dma_start(extr_sb[:], extr_d[:, :])
            nc.gpsimd.dma_start(sqx_sb[:], sqx_d[:, :])
            nc.gpsimd.dma_start(uc_sb[:], uc_d[:, :])

            # --- warmup + j-row broadcasts (quarters 2-3 only) ---
            for w in range(N_WARMUP):
                wps = psum_p.tile([128, 1024], dt.float32, tag="pe",
                                  name=f"warm{w}")
                nc.tensor.matmul(
                    wps[:, 0:256],
                    lhsT=warm_sb[:, :, 0:128],
                    rhs=warm_sb[:, :, :],
                    start=True, stop=True,
                    perf_mode=PM.DoubleRow,
                )
            bcv = psum_p.tile([128, 1024], dt.float32, tag="dv", name="bcv")
            for q in range(2):
                nc.tensor.matmul(
                    bcv[:, q * 512:(q + 1) * 512],
                    lhsT=ones_sb[:, :],
                    rhs=vrow_sb[:, 1024 + q * 512:1024 + (q + 1) * 512],
                    start=True, stop=True,
                )
            bcb = psum_p.tile([128, 1024], dt.float32, tag="dv", name="bcb")
            for q in range(2):
                nc.tensor.matmul(
                    bcb[:, q * 512:(q + 1) * 512],
                    lhsT=ones_sb[:, :],
                    rhs=brow_sb[:, 1024 + q * 512:1024 + (q + 1) * 512],
                    start=True, stop=True,
                )
            nc.vector.tensor_copy(v_rep[:], bcv[:])
            nc.scalar.copy(B_rep[:], bcb[:])

            # --- T1[po] = v_rep * sqx_po + B_rep (DVE-only op, ring) ---
            def emit_t1(po):
                nc.vector.scalar_tensor_tensor(
                    t1_sb[:, po % RING, :],
                    v_rep[:], sqx_sb[:, po:po + 1], B_rep[:],
                    OP.mult, OP.add,
                )

            warm = 1
            emit_t1(0)

            for po in range(PO):
                pog, pi = divmod(po, 4)
                lhsT = x8_sb[:, :, pog, :, pi * 128:(pi + 1) * 128]
                ot = stage.tile([128, MS], dt.bfloat16, tag="ot",
                                name=f"ot{po}")
                if not const_mask:
                    rt = stage.tile([128, MS], dt.float32, tag="rt",
                                    name=f"rt{po}")
                last = po == PO - 1
                uc = uc_sb[:, po:po + 1]

                def epi(dst0, src_t, width):
                    nhl = 2 if last else 1
                    wl = width // nhl
                    for h in range(nhl):
                        dsl = slice(dst0 + h * wl, dst0 + (h + 1) * wl)
                        ssl = slice(h * wl, (h + 1) * wl)
                        if const_mask:
                            _act_recip(nc, ot[:, dsl], src_t[:, ssl], uc, bias)
                        else:
                            _act_recip(nc, rt[:, dsl], src_t[:, ssl], uc, bias)
                            nc.vector.scalar_tensor_tensor(
                                ot[:, dsl], rt[:, dsl], c0, rt[:, dsl],
                                OP.add, OP.mult,
                            )
                        if last:
                            nc.sync.dma_start(out_v[:, po, dsl], ot[:, dsl])

                # pe tile: quarters 0-1 (DR passes then ext passes)
                pe_t = psum_p.tile([128, 1024], dt.float32, tag="pe",
                                   name=f"pe{po}")
                for kt2 in range(2):
                    for jt in range(2):
                        nc.tensor.matmul(
                            pe_t[:, jt * 512:(jt + 1) * 512],
                            lhsT=lhsT[:, kt2],
                            rhs=y8_sb[:, kt2, jt],
                            start=(kt2 == 0), stop=False,
                            perf_mode=PM.DoubleRow,
                        )
                for jt in range(2):
                    nc.tensor.matmul(
                        pe_t[:, jt * 512:(jt + 1) * 512],
                        lhsT=extl_sb[:, po * 128:(po + 1) * 128],
                        rhs=extr_sb[:, jt * 512:(jt + 1) * 512],
                        start=False, stop=True,
                    )
                epi(0, pe_t, 1024)

                # dv tile: quarters 2-3 (DR passes then DVE T1 add)
                dv_t = psum_p.tile([128, 1024], dt.float32, tag="dv",
                                   name=f"dv{po}")
                for kt2 in range(2):
                    for jt in range(2, 4):
                        nc.tensor.matmul(
                            dv_t[:, (jt - 2) * 512:(jt - 1) * 512],
                            lhsT=lhsT[:, kt2],
                            rhs=y8_sb[:, kt2, jt],
                            start=(kt2 == 0), stop=(kt2 == 1),
                            perf_mode=PM.DoubleRow,
                        )
                nc.vector.tensor_tensor(
                    dv_t[:], t1_sb[:, po % RING, :], dv_t[:], OP.add,
                )
                epi(1024, dv_t, 1024)

                if po + warm < PO:
                    emit_t1(po + warm)
                if not last:
                    nc.sync.dma_start(out_v[:, po, :], ot[:, :])

    nc.compile()
    return nc


def _hi_lo(vec, bf16):
    hi = vec.astype(bf16)
    lo = (vec - hi.astype(np.float64)).astype(bf16)
    return hi, lo


def kernel(x, y, sample_x, sample_y, scale, cutoff, phi):
    import ml_dtypes
    from concourse.bass_utils import run_bass_kernel_spmd

    bf16 = ml_dtypes.bfloat16
    fp8 = ml_dtypes.float8_e4m3

    phi_val = float(np.asarray(phi).reshape(-1)[0])
    cutoff_val = float(np.clip(np.asarray(cutoff).reshape(-1)[0], 0.0, 1000.0))

    x64 = np.asarray(x, dtype=np.float32).astype(np.float64)
    y64 = np.asarray(y, dtype=np.float32).astype(np.float64)
    sc64 = np.asarray(scale, dtype=np.float32).astype(np.float64).reshape(-1)
    sx64 = np.asarray(sample_x, dtype=np.float32).astype(np.float64)
    sy64 = np.asarray(sample_y, dtype=np.float32).astype(np.float64)

    u = np.clip(np.log1p(np.exp(sx64 @ sc64)), SOFTPLUS_MIN, SOFTPLUS_MAX) ** -0.5
    v = np.clip(np.log1p(np.exp(sy64 @ sc64)), SOFTPLUS_MIN, SOFTPLUS_MAX) ** -0.5
    sqx = (x64 * x64).sum(axis=1)  # [N]
    sqy = (y64 * y64).sum(axis=1)  # [M]

    # res range from a subsample -> mask fit interval
    rng = np.random.default_rng(12345)
    ii = rng.integers(0, N, 4096)
    jj = rng.integers(0, M, 4096)
    dd = sqx[ii] + sqy[jj] - 2.0 * np.einsum("nd,nd->n", x64[ii], y64[jj])
    res_s = 1.0 / (1.0 + dd * (u[ii] * v[jj]))
    R = float(min(1.0, max(2.0 * res_s.max(), 0.01)))

    m0, m1, gerr, c_const, const_err = _fit_mask(phi_val, cutoff_val, R)
    const_mask = const_err < 6e-3
    if const_mask:
        cc = c_const
        c0 = 0.0
    else:
        assert gerr < 2e-3, f"mask linearization too coarse: {gerr}"
        cc = float(np.sqrt(m1))
        c0 = m0 / cc
    bias = 1.0 / cc
    sc_p = u / (PSC * cc)   # per-partition ACT scale over N

    key = (const_mask, round(c0, 9), round(bias, 9), RING)
    if key not in _CACHE:
        _CACHE[key] = _build(const_mask, c0, bias)
    nc = _CACHE[key]

    x8T = (GX * x64).T.astype(fp8)                  # [D, N]
    y8T = (GY * (v[:, None] * y64)).T.astype(fp8)   # [D, M]
    wv = PSC * v              # [M]
    brw = PSC * v * sqy       # [M]
    sqxh, sqxl = _hi_lo(sqx, bf16)
    wvh, wvl = _hi_lo(wv, bf16)
    bh, bl = _hi_lo(brw, bf16)
    extL = np.stack([sqxh, sqxh, sqxl,
                     np.ones(N, dtype=bf16), np.ones(N, dtype=bf16)])
    extR = np.stack([wvh, wvl, wvh, bh, bl])
    ones_row = np.ones((2, 128), dtype=bf16)

    in_maps = []
    for c in range(CORES):
        cx, cy = divmod(c, YB)
        si, sj = cx * NS, cy * MS
        xt = x8T[:, si:si + NS].reshape(2, 2, 128, 4, 512)  # [kt2,ktp,p,g,c]
        yt = y8T[:, sj:sj + MS].reshape(2, 2, 128, 4, 512)
        in_maps.append(
            {
                "x8T_shard": np.ascontiguousarray(
                    xt.transpose(2, 0, 3, 1, 4)).reshape(128, KT * NS),
                "y8T_shard": np.ascontiguousarray(
                    yt.transpose(2, 0, 3, 1, 4)).reshape(128, KT * MS),
                "sqx_shard": np.ascontiguousarray(
                    sqx[si:si + NS].reshape(PO, 128).T.astype(np.float32)),
                "uc_shard": np.ascontiguousarray(
                    sc_p[si:si + NS].reshape(PO, 128).T.astype(np.float32)),
                "extL_shard": np.ascontiguousarray(extL[:, si:si + NS]),
                "extR_shard": np.ascontiguousarray(extR[:, sj:sj + MS]),
                "vrow_shard": np.ascontiguousarray(
                    np.stack(_hi_lo(wv[sj:sj + MS], bf16))),
                "brow_shard": np.ascontiguousarray(
                    np.stack(_hi_lo(brw[sj:sj + MS], bf16))),
                "ones_row": ones_row,
            }
        )

    trace = bool(int(os.environ.get("KERNEL_TRACE", "0")))
    r = run_bass_kernel_spmd(nc, in_maps, core_ids=list(range(CORES)), trace=trace)
    kernel.last_results = r
    out = np.empty((N, M), dtype=np.float32)
    for c in range(CORES):
        cx, cy = divmod(c, YB)
        out[cx * NS:(cx + 1) * NS, cy * MS:(cy + 1) * MS] = np.asarray(
            r.results[c]["out_shard"]
        ).astype(np.float32)
    return out


if __name__ == "__main__":
    rng = np.random.default_rng(0)
    ins = {
        "x": rng.standard_normal((N, D), dtype=np.float32),
        "y": rng.standard_normal((M, D), dtype=np.float32),
        "sample_x": rng.random((N, S), dtype=np.float32),
        "sample_y": rng.random((M, S), dtype=np.float32),
        "scale": rng.random((S,), dtype=np.float32),
        "cutoff": np.full((1,), 0.1, dtype=np.float32),
        "phi": np.ones((1,), dtype=np.float32),
    }
    o = kernel(**ins)
    print(o.shape, o.dtype, o[:2, :4])
